# revision 26
# baseline (speedup 1.0000x reference)
"""Trainium2 Bass kernel for nn_CrossAttention (single-query cross attention).

Reference computation (B=4, C=64, H=W=128, heads h=64, dim_head d=64,
inner=4096, HW=16384):
    x[b, j, c]   = fimg[b, c, j]                       (j indexes H*W)
    q[b, h, d]   = sum_e fpsf[b, e] Wq[h*64+d, e]
    k[b, j, h, d]= sum_c x[b, j, c] Wk[h*64+d, c]
    out[b, h, j] = scale * sum_d q[b,h,d] k[b,j,h,d]

Single query per (batch, head) collapses the attention:
    W2[b, h, c]  = scale * sum_d q[b,h,d] Wk[h*64+d, c]      (tiny)
    out[b, h, j] = sum_c W2[b,h,c] fimg[b, c, j]
a 64x FLOP reduction vs materializing k.

Sharding: j (H*W = 16384) split across 8 cores (2048 each); every core
redundantly computes W2.

v2 layout (all zero-padding moved into tiny on-chip tensors; weights are
dense in HBM):
  WqF2  [128, 2056] bf16: cols 0:8 = fpsf expanded block-diag over the
        two 64-e halves (host-built zeros), cols 8: = Wq.T chunk pairs
        stacked along partitions (rows 0:64 = even 128-col chunks,
        rows 64:128 = odd chunks) -> step A is 16 matmuls of N=8.
  Wk_st [128, 2048] bf16: per head-pair p, cols 64p:64p+64 stack
        Wk_{2p}[d,c] (rows 0:64) over Wk_{2p+1}[d,c] (rows 64:128).
        Step B rhs is a zero-expanded q2T built on-chip (4 DVE copies),
        so no 2x zero padding travels over HBM.
  fimg_s [256, 2048] bf16, out [256, 2048] bf16 (host casts to f32).

DMA: two HWDGE queues (sync + scalar engines), ~305 GB/s each observed;
inputs split across them so the W2 weight chain and fimg stream overlap.
9 dummy matmuls warm the PE clock gate (HAM) during the initial DMA wait
so the real matmuls run at 2.4 GHz instead of 1.2.

Baseline (single sync queue, f32 out, block-diag Wk): ~29.4us.
"""

import sys
import types

import numpy as np
import ml_dtypes

# antenv.axon_hooks is absent in this image; bass_utils imports it when
# tracing. Register a minimal stand-in before importing concourse.
if "antenv.axon_hooks" not in sys.modules:
    try:
        import antenv  # noqa: F401

        _hooks = types.ModuleType("antenv.axon_hooks")
        _hooks._hook = None

        def _set_hook(h):
            _hooks._hook = h

        _hooks.set_axon_ntff_profile_hook = _set_hook
        _hooks.get_axon_ntff_profile_hook = lambda: _hooks._hook
        sys.modules["antenv.axon_hooks"] = _hooks
        try:
            from trn_agent_boot.trn_boot import _ntff_profile_via_ctypes

            _set_hook(_ntff_profile_via_ctypes("/opt/axon/libaxon_pjrt.so"))
        except Exception:
            pass
    except ImportError:
        pass

import concourse.bass as bass  # noqa: E402
import concourse.mybir as mybir  # noqa: E402
import concourse.tile as tile  # noqa: E402
from concourse import bacc  # noqa: E402
from concourse.bass_utils import run_bass_kernel_spmd  # noqa: E402

N_CORES = 8
B, C, H, W = 4, 64, 128, 128
HEADS, DIM_HEAD = 64, 64
HW = H * W
JS = HW // N_CORES  # 2048 j-positions per core
SCALE = DIM_HEAD ** -0.5
F32 = mybir.dt.float32
BF16 = mybir.dt.bfloat16
NPBF16 = ml_dtypes.bfloat16

_compiled = None  # cache (nc) across calls


def _build():
    nc = bacc.Bacc("TRN2", target_bir_lowering=False, debug=False,
                   num_devices=N_CORES)

    fimg_d = nc.dram_tensor("fimg_s", [2 * 128, JS], BF16, kind="ExternalInput")
    wqf2_d = nc.dram_tensor("WqF2", [128, 2056], BF16, kind="ExternalInput")
    wk_d = nc.dram_tensor("Wk_st", [128, 2048], BF16, kind="ExternalInput")
    out_d = nc.dram_tensor("out", [2 * 128, JS], BF16, kind="ExternalOutput")

    with tile.TileContext(nc) as tc:
        with (
            tc.tile_pool(name="weights", bufs=1) as wpool,
            tc.tile_pool(name="img", bufs=1) as ipool,
            tc.tile_pool(name="small_ps", bufs=1, space="PSUM") as spsum,
            tc.tile_pool(name="big_ps", bufs=6, space="PSUM") as bpsum,
            tc.tile_pool(name="ostage", bufs=1) as opool,
        ):
            # Zero-init tiles: no input deps, runs during the NEFF entry
            # sequence / first DMA wait. All on gpsimd (SBUF-only engine,
            # otherwise idle); the one-time cross-engine WAW chain
            # resolves long before the readers run.
            q2b = wpool.tile([128, 256], BF16, tag="q2b")
            nc.gpsimd.memset(q2b[:], 0.0)
            bds = []
            for q in range(2):
                bd = wpool.tile([128, 128], BF16, tag=f"bd{q}")
                nc.gpsimd.memset(bd[:], 0.0)
                bds.append(bd)

            # Input DMAs on two HWDGE queues (sync + scalar), which share
            # ~295 GB/s aggregate. All transfers keep 4KB+ per-partition
            # rows (smaller rows tank packet throughput). The two weight
            # tensors are split by partition halves across BOTH queues so
            # the W2 chain's inputs land as early as possible.
            wqf2 = wpool.tile([128, 2056], BF16, tag="wqf2")
            nc.sync.dma_start(wqf2[:], wqf2_d.ap())
            wk = wpool.tile([128, 2048], BF16, tag="wk")
            nc.scalar.dma_start(wk[:], wk_d.ap())
            imgs = []
            for q in range(2):
                img_t = ipool.tile([128, JS], BF16, tag=f"img{q}")
                eng = nc.sync if q == 0 else nc.scalar
                eng.dma_start(img_t[:], fimg_d.ap()[128 * q:128 * (q + 1), :])
                imgs.append(img_t)

            # A: q2T[r, 4m+b] = q[b, 128m + r]; chunk pairs m=2p,2p+1
            # stacked along partitions, fpsf block-diag-expanded -> N=8.
            # Two separate PSUM tiles (t = m//16) so the expansion for the
            # first half doesn't wait for all of A (deps are tracked at
            # tile granularity).
            fpsf_exp = wqf2[:, 0:8]
            q2T_ps = spsum.tile([128, 128], F32, tag="q2T_ps")
            for p in range(16):
                nc.tensor.matmul(
                    q2T_ps[:, 8 * p:8 * p + 8],
                    wqf2[:, 8 + 128 * p:8 + 128 * p + 128],
                    fpsf_exp,
                    start=True, stop=True,
                )

            # Zero-expand q2T for B (scale folded in): col 8p+2b+par
            # holds q[b, head 2p+par] in rows 64par:64par+64, zeros
            # elsewhere. Two strided ops, both on vector: pushing any of
            # these to the scalar engine measures ~3us slower end-to-end
            # (scheduler reordering + handoff latency on the chain).
            for par in range(2):
                dst = q2b[64 * par:64 * par + 64, par:256:2]
                src = q2T_ps[64 * par:64 * par + 64, :]
                nc.vector.tensor_scalar_mul(dst, src, SCALE)

            # B: w2_ps[c, 8p + 2b + par] = scale * W2[b, 2p+par, c]
            w2_ps = spsum.tile([64, 256], F32, tag="w2_ps")
            for p in range(32):
                nc.tensor.matmul(
                    w2_ps[:, 8 * p:8 * p + 8],
                    wk[:, 64 * p:64 * p + 64],
                    q2b[:, 8 * p:8 * p + 8],
                    start=True, stop=True,
                )

            # Assembly: bd_q[64*half + c, 64*half + 2p+par] =
            # sW2[2q+half, 2p+par, c]. One merged 3D-AP copy per (q,
            # half): src w2[c, 8p + 2b + par], dst bd[c, 2p + par].
            # bd0 on scalar (idle at this point), bd1 on vector — one
            # writer engine per tile, both start right after B.
            w2_3d = w2_ps[:, :].rearrange("c (p x) -> c p x", p=32)
            for q in range(2):
                for half in range(2):
                    b = 2 * q + half
                    dst = bds[q][64 * half:64 * half + 64,
                                 64 * half:64 * half + 64].rearrange(
                                     "c (p x) -> c p x", p=32)
                    src = w2_3d[:, :, 2 * b:2 * b + 2]
                    if q == 0:
                        nc.scalar.copy(dst, src)
                    else:
                        nc.vector.tensor_copy(dst, src)

            # Big: out rows pair q = bd_q.T @ img_q in 512-col chunks.
            # Stage per (q, half) into [128, 1024] bf16 tiles, each with a
            # single writer engine (h0 -> vector, h1 -> scalar), and DMA
            # each half as soon as its two casts land (4 out DMAs total,
            # 2KB rows, alternating queues).
            for q in range(2):
                for half in range(2):
                    ot = opool.tile([128, JS // 2], BF16, tag=f"ot{q}{half}")
                    for kk in range(2):
                        k = 2 * half + kk
                        ps = bpsum.tile([128, 512], F32, tag="mm_ps")
                        nc.tensor.matmul(
                            ps[:], bds[q][:],
                            imgs[q][:, 512 * k:512 * k + 512],
                            start=True, stop=True,
                        )
                        dst = ot[:, 512 * kk:512 * kk + 512]
                        if half == 0:
                            nc.vector.tensor_copy(dst, ps[:])
                        else:
                            nc.scalar.copy(dst, ps[:])
                    # 3rd output channel: gpsimd SWDGE takes q0h1 so the
                    # scalar HWDGE queue is free for the last chunk.
                    eng = {(0, 0): nc.sync, (0, 1): nc.gpsimd,
                           (1, 0): nc.sync, (1, 1): nc.scalar}[(q, half)]
                    eng.dma_start(
                        out_d.ap()[128 * q:128 * (q + 1),
                                   1024 * half:1024 * half + 1024],
                        ot[:])

    nc.compile()
    return nc


def _prep_inputs(fpsf, fimg, Wq, Wk):
    fpsf = np.ascontiguousarray(fpsf, dtype=np.float32)
    fimg = np.ascontiguousarray(fimg, dtype=np.float32)
    Wq = np.ascontiguousarray(Wq, dtype=np.float32)
    Wk = np.ascontiguousarray(Wk, dtype=np.float32)

    WqT3 = Wq.T.reshape(64, 32, 128)  # [e, chunk, s]
    wqf2 = np.zeros((128, 2056), np.float32)
    wqf2[0:64, 0:4] = fpsf.T
    wqf2[64:128, 4:8] = fpsf.T
    wqf2[0:64, 8:] = WqT3[:, 0::2, :].reshape(64, 2048)
    wqf2[64:128, 8:] = WqT3[:, 1::2, :].reshape(64, 2048)
    WqF2 = wqf2.astype(NPBF16)

    Wk3 = Wk.reshape(64, 64, 64)  # [h, d, c]
    wk = np.empty((128, 2048), np.float32)
    wk[0:64] = Wk3[0::2].transpose(1, 0, 2).reshape(64, 2048)
    wk[64:128] = Wk3[1::2].transpose(1, 0, 2).reshape(64, 2048)
    Wk_st = wk.astype(NPBF16)

    fimg_f = fimg.reshape(B, C, HW).astype(NPBF16)
    in_maps = []
    for i in range(N_CORES):
        sh = np.ascontiguousarray(
            fimg_f[:, :, JS * i:JS * (i + 1)]).reshape(2 * 128, JS)
        in_maps.append({
            "fimg_s": sh,
            "WqF2": WqF2,
            "Wk_st": Wk_st,
        })
    return in_maps


def kernel(fpsf, fimg, Wq, Wk):
    global _compiled
    if _compiled is None:
        _compiled = _build()
    nc = _compiled

    in_maps = _prep_inputs(fpsf, fimg, Wq, Wk)
    res = run_bass_kernel_spmd(nc, in_maps, core_ids=list(range(N_CORES)))

    out = np.empty((B, HEADS, HW), dtype=np.float32)
    for i in range(N_CORES):
        out[:, :, JS * i:JS * (i + 1)] = \
            res.results[i]["out"].astype(np.float32).reshape(B, HEADS, JS)
    return out.reshape(B, C, H, W)


if __name__ == "__main__":
    rng = np.random.default_rng(0)
    ins = {
        "fpsf": rng.standard_normal((B, C), dtype=np.float32),
        "fimg": rng.standard_normal((B, C, H, W), dtype=np.float32),
        "Wq": (rng.standard_normal((4096, C), dtype=np.float32) * 0.05),
        "Wk": (rng.standard_normal((4096, C), dtype=np.float32) * 0.05),
    }
    out = kernel(**ins)
    print("out", out.shape, out.dtype, float(np.abs(out).max()))


# revision 30
# speedup vs baseline: 1.0125x; 1.0125x over previous
"""Trainium2 Bass kernel for nn_CrossAttention (single-query cross attention).

Reference computation (B=4, C=64, H=W=128, heads h=64, dim_head d=64,
inner=4096, HW=16384):
    x[b, j, c]   = fimg[b, c, j]                       (j indexes H*W)
    q[b, h, d]   = sum_e fpsf[b, e] Wq[h*64+d, e]
    k[b, j, h, d]= sum_c x[b, j, c] Wk[h*64+d, c]
    out[b, h, j] = scale * sum_d q[b,h,d] k[b,j,h,d]

Single query per (batch, head) collapses the attention:
    W2[b, h, c]  = scale * sum_d q[b,h,d] Wk[h*64+d, c]      (tiny)
    out[b, h, j] = sum_c W2[b,h,c] fimg[b, c, j]
a 64x FLOP reduction vs materializing k.

Sharding: j (H*W = 16384) split across 8 cores (2048 each); every core
redundantly computes W2.

v2 layout (all zero-padding moved into tiny on-chip tensors; weights are
dense in HBM):
  WqF2  [128, 2056] bf16: cols 0:8 = fpsf expanded block-diag over the
        two 64-e halves (host-built zeros), cols 8: = Wq.T chunk pairs
        stacked along partitions (rows 0:64 = even 128-col chunks,
        rows 64:128 = odd chunks) -> step A is 16 matmuls of N=8.
  Wk_st [128, 2048] bf16: per head-pair p, cols 64p:64p+64 stack
        Wk_{2p}[d,c] (rows 0:64) over Wk_{2p+1}[d,c] (rows 64:128).
        Step B rhs is a zero-expanded q2T built on-chip (4 DVE copies),
        so no 2x zero padding travels over HBM.
  fimg_s [256, 2048] bf16, out [256, 2048] bf16 (host casts to f32).

DMA: two HWDGE queues (sync + scalar engines), ~305 GB/s each observed;
inputs split across them so the W2 weight chain and fimg stream overlap.
9 dummy matmuls warm the PE clock gate (HAM) during the initial DMA wait
so the real matmuls run at 2.4 GHz instead of 1.2.

Baseline (single sync queue, f32 out, block-diag Wk): ~29.4us.
"""

import sys
import types

import numpy as np
import ml_dtypes

# antenv.axon_hooks is absent in this image; bass_utils imports it when
# tracing. Register a minimal stand-in before importing concourse.
if "antenv.axon_hooks" not in sys.modules:
    try:
        import antenv  # noqa: F401

        _hooks = types.ModuleType("antenv.axon_hooks")
        _hooks._hook = None

        def _set_hook(h):
            _hooks._hook = h

        _hooks.set_axon_ntff_profile_hook = _set_hook
        _hooks.get_axon_ntff_profile_hook = lambda: _hooks._hook
        sys.modules["antenv.axon_hooks"] = _hooks
        try:
            from trn_agent_boot.trn_boot import _ntff_profile_via_ctypes

            _set_hook(_ntff_profile_via_ctypes("/opt/axon/libaxon_pjrt.so"))
        except Exception:
            pass
    except ImportError:
        pass

import concourse.bass as bass  # noqa: E402
import concourse.mybir as mybir  # noqa: E402
import concourse.tile as tile  # noqa: E402
from concourse import bacc  # noqa: E402
from concourse.bass_utils import run_bass_kernel_spmd  # noqa: E402

N_CORES = 8
B, C, H, W = 4, 64, 128, 128
HEADS, DIM_HEAD = 64, 64
HW = H * W
JS = HW // N_CORES  # 2048 j-positions per core
SCALE = DIM_HEAD ** -0.5
F32 = mybir.dt.float32
BF16 = mybir.dt.bfloat16
NPBF16 = ml_dtypes.bfloat16

_compiled = None  # cache (nc) across calls


def _build():
    nc = bacc.Bacc("TRN2", target_bir_lowering=False, debug=False,
                   num_devices=N_CORES)

    fimg_d = nc.dram_tensor("fimg_s", [2 * 128, JS], BF16, kind="ExternalInput")
    wqf2_d = nc.dram_tensor("WqF2", [128, 2056], BF16, kind="ExternalInput")
    wk_d = nc.dram_tensor("Wk_st", [128, 2048], BF16, kind="ExternalInput")
    out_d = nc.dram_tensor("out", [2 * 128, JS], BF16, kind="ExternalOutput")

    with tile.TileContext(nc) as tc:
        with (
            tc.tile_pool(name="weights", bufs=1) as wpool,
            tc.tile_pool(name="img", bufs=1) as ipool,
            tc.tile_pool(name="small_ps", bufs=1, space="PSUM") as spsum,
            tc.tile_pool(name="big_ps", bufs=6, space="PSUM") as bpsum,
            tc.tile_pool(name="ostage", bufs=1) as opool,
        ):
            # Zero-init tiles: no input deps, runs during the NEFF entry
            # sequence / first DMA wait. All on gpsimd (SBUF-only engine,
            # otherwise idle); the one-time cross-engine WAW chain
            # resolves long before the readers run.
            q2b = []
            for t in range(2):
                tt = wpool.tile([128, 128], BF16, tag=f"q2b{t}")
                nc.gpsimd.memset(tt[:], 0.0)
                q2b.append(tt)
            bds = []
            for q in range(2):
                bd = wpool.tile([128, 128], BF16, tag=f"bd{q}")
                nc.gpsimd.memset(bd[:], 0.0)
                bds.append(bd)

            # Input DMAs on two HWDGE queues (sync + scalar), which share
            # ~295 GB/s aggregate. All transfers keep 4KB+ per-partition
            # rows (smaller rows tank packet throughput). The two weight
            # tensors are split by partition halves across BOTH queues so
            # the W2 chain's inputs land as early as possible.
            wqf2 = wpool.tile([128, 2056], BF16, tag="wqf2")
            nc.sync.dma_start(wqf2[:], wqf2_d.ap())
            wk = wpool.tile([128, 2048], BF16, tag="wk")
            nc.scalar.dma_start(wk[:], wk_d.ap())
            imgs = []
            for q in range(2):
                img_t = ipool.tile([128, JS], BF16, tag=f"img{q}")
                eng = nc.sync if q == 0 else nc.scalar
                eng.dma_start(img_t[:], fimg_d.ap()[128 * q:128 * (q + 1), :])
                imgs.append(img_t)

            # A: q2T[r, 4m+b] = q[b, 128m + r]; chunk pairs m=2p,2p+1
            # stacked along partitions, fpsf block-diag-expanded -> N=8.
            # Two separate PSUM tiles (t = m//16) so the expansion for the
            # first half doesn't wait for all of A (deps are tracked at
            # tile granularity).
            fpsf_exp = wqf2[:, 0:8]
            q2T_ps = spsum.tile([128, 128], F32, tag="q2T_ps")
            for p in range(16):
                nc.tensor.matmul(
                    q2T_ps[:, 8 * p:8 * p + 8],
                    wqf2[:, 8 + 128 * p:8 + 128 * p + 128],
                    fpsf_exp,
                    start=True, stop=True,
                )

            # Zero-expand q2T for B (scale folded in): col 8pp+2b+par
            # holds q[b, head 2p+par] in rows 64par:64par+64, zeros
            # elsewhere. All on vector (scalar on this chain measures
            # ~3us slower: scheduler reordering + handoff latency); two
            # tiles so B's first half only waits for the t=0 ops.
            for t in range(2):
                for par in range(2):
                    dst = q2b[t][64 * par:64 * par + 64, par:128:2]
                    src = q2T_ps[64 * par:64 * par + 64, 64 * t:64 * t + 64]
                    nc.vector.tensor_scalar_mul(dst, src, SCALE)

            # B: w2_ps[c, 8p + 2b + par] = scale * W2[b, 2p+par, c]
            w2_ps = spsum.tile([64, 256], F32, tag="w2_ps")
            for p in range(32):
                t, pp = divmod(p, 16)
                nc.tensor.matmul(
                    w2_ps[:, 8 * p:8 * p + 8],
                    wk[:, 64 * p:64 * p + 64],
                    q2b[t][:, 8 * pp:8 * pp + 8],
                    start=True, stop=True,
                )

            # Assembly: bd_q[64*half + c, 64*half + 2p+par] =
            # sW2[2q+half, 2p+par, c]. One merged 3D-AP copy per (q,
            # half): src w2[c, 8p + 2b + par], dst bd[c, 2p + par].
            # bd0 on scalar (idle at this point), bd1 on vector — one
            # writer engine per tile, both start right after B.
            w2_3d = w2_ps[:, :].rearrange("c (p x) -> c p x", p=32)
            for q in range(2):
                for half in range(2):
                    b = 2 * q + half
                    dst = bds[q][64 * half:64 * half + 64,
                                 64 * half:64 * half + 64].rearrange(
                                     "c (p x) -> c p x", p=32)
                    src = w2_3d[:, :, 2 * b:2 * b + 2]
                    if q == 0:
                        nc.scalar.copy(dst, src)
                    else:
                        nc.vector.tensor_copy(dst, src)

            # Big: out rows pair q = bd_q.T @ img_q in 512-col chunks.
            # Stage per (q, half) into [128, 1024] bf16 tiles, each with a
            # single writer engine (h0 -> vector, h1 -> scalar), and DMA
            # each half as soon as its two casts land (4 out DMAs total,
            # 2KB rows, alternating queues).
            for q in range(2):
                for half in range(2):
                    ot = opool.tile([128, JS // 2], BF16, tag=f"ot{q}{half}")
                    for kk in range(2):
                        k = 2 * half + kk
                        ps = bpsum.tile([128, 512], F32, tag="mm_ps")
                        nc.tensor.matmul(
                            ps[:], bds[q][:],
                            imgs[q][:, 512 * k:512 * k + 512],
                            start=True, stop=True,
                        )
                        dst = ot[:, 512 * kk:512 * kk + 512]
                        if half == 0:
                            nc.vector.tensor_copy(dst, ps[:])
                        else:
                            nc.scalar.copy(dst, ps[:])
                    (nc.sync if half == 0 else nc.scalar).dma_start(
                        out_d.ap()[128 * q:128 * (q + 1),
                                   1024 * half:1024 * half + 1024],
                        ot[:])

    nc.compile()
    return nc


def _prep_inputs(fpsf, fimg, Wq, Wk):
    fpsf = np.ascontiguousarray(fpsf, dtype=np.float32)
    fimg = np.ascontiguousarray(fimg, dtype=np.float32)
    Wq = np.ascontiguousarray(Wq, dtype=np.float32)
    Wk = np.ascontiguousarray(Wk, dtype=np.float32)

    WqT3 = Wq.T.reshape(64, 32, 128)  # [e, chunk, s]
    wqf2 = np.zeros((128, 2056), np.float32)
    wqf2[0:64, 0:4] = fpsf.T
    wqf2[64:128, 4:8] = fpsf.T
    wqf2[0:64, 8:] = WqT3[:, 0::2, :].reshape(64, 2048)
    wqf2[64:128, 8:] = WqT3[:, 1::2, :].reshape(64, 2048)
    WqF2 = wqf2.astype(NPBF16)

    Wk3 = Wk.reshape(64, 64, 64)  # [h, d, c]
    wk = np.empty((128, 2048), np.float32)
    wk[0:64] = Wk3[0::2].transpose(1, 0, 2).reshape(64, 2048)
    wk[64:128] = Wk3[1::2].transpose(1, 0, 2).reshape(64, 2048)
    Wk_st = wk.astype(NPBF16)

    fimg_f = fimg.reshape(B, C, HW).astype(NPBF16)
    in_maps = []
    for i in range(N_CORES):
        sh = np.ascontiguousarray(
            fimg_f[:, :, JS * i:JS * (i + 1)]).reshape(2 * 128, JS)
        in_maps.append({
            "fimg_s": sh,
            "WqF2": WqF2,
            "Wk_st": Wk_st,
        })
    return in_maps


def kernel(fpsf, fimg, Wq, Wk):
    global _compiled
    if _compiled is None:
        _compiled = _build()
    nc = _compiled

    in_maps = _prep_inputs(fpsf, fimg, Wq, Wk)
    res = run_bass_kernel_spmd(nc, in_maps, core_ids=list(range(N_CORES)))

    out = np.empty((B, HEADS, HW), dtype=np.float32)
    for i in range(N_CORES):
        out[:, :, JS * i:JS * (i + 1)] = \
            res.results[i]["out"].astype(np.float32).reshape(B, HEADS, JS)
    return out.reshape(B, C, H, W)


if __name__ == "__main__":
    rng = np.random.default_rng(0)
    ins = {
        "fpsf": rng.standard_normal((B, C), dtype=np.float32),
        "fimg": rng.standard_normal((B, C, H, W), dtype=np.float32),
        "Wq": (rng.standard_normal((4096, C), dtype=np.float32) * 0.05),
        "Wk": (rng.standard_normal((4096, C), dtype=np.float32) * 0.05),
    }
    out = kernel(**ins)
    print("out", out.shape, out.dtype, float(np.abs(out).max()))


# revision 35
# speedup vs baseline: 1.0702x; 1.0570x over previous
"""Trainium2 Bass kernel for nn_CrossAttention (single-query cross attention).

Reference computation (B=4, C=64, H=W=128, heads h=64, dim_head d=64,
inner=4096, HW=16384):
    x[b, j, c]   = fimg[b, c, j]                       (j indexes H*W)
    q[b, h, d]   = sum_e fpsf[b, e] Wq[h*64+d, e]
    k[b, j, h, d]= sum_c x[b, j, c] Wk[h*64+d, c]
    out[b, h, j] = scale * sum_d q[b,h,d] k[b,j,h,d]

Single query per (batch, head) collapses the attention:
    W2[b, h, c]  = scale * sum_d q[b,h,d] Wk[h*64+d, c]      (tiny)
    out[b, h, j] = sum_c W2[b,h,c] fimg[b, c, j]
a 64x FLOP reduction vs materializing k.

Sharding: j (H*W = 16384) split across 8 cores (2048 each); every core
redundantly computes W2.

v2 layout (all zero-padding moved into tiny on-chip tensors; weights are
dense in HBM):
  WqF2  [128, 2056] bf16: cols 0:8 = fpsf expanded block-diag over the
        two 64-e halves (host-built zeros), cols 8: = Wq.T chunk pairs
        stacked along partitions (rows 0:64 = even 128-col chunks,
        rows 64:128 = odd chunks) -> step A is 16 matmuls of N=8.
  Wk_st [128, 2048] bf16: per head-pair p, cols 64p:64p+64 stack
        Wk_{2p}[d,c] (rows 0:64) over Wk_{2p+1}[d,c] (rows 64:128).
        Step B rhs is a zero-expanded q2T built on-chip (4 DVE copies),
        so no 2x zero padding travels over HBM.
  fimg_s [256, 2048] bf16, out [256, 2048] bf16 (host casts to f32).

DMA: two HWDGE queues (sync + scalar engines), ~305 GB/s each observed;
inputs split across them so the W2 weight chain and fimg stream overlap.
9 dummy matmuls warm the PE clock gate (HAM) during the initial DMA wait
so the real matmuls run at 2.4 GHz instead of 1.2.

Baseline (single sync queue, f32 out, block-diag Wk): ~29.4us.
"""

import sys
import types

import numpy as np
import ml_dtypes

# antenv.axon_hooks is absent in this image; bass_utils imports it when
# tracing. Register a minimal stand-in before importing concourse.
if "antenv.axon_hooks" not in sys.modules:
    try:
        import antenv  # noqa: F401

        _hooks = types.ModuleType("antenv.axon_hooks")
        _hooks._hook = None

        def _set_hook(h):
            _hooks._hook = h

        _hooks.set_axon_ntff_profile_hook = _set_hook
        _hooks.get_axon_ntff_profile_hook = lambda: _hooks._hook
        sys.modules["antenv.axon_hooks"] = _hooks
        try:
            from trn_agent_boot.trn_boot import _ntff_profile_via_ctypes

            _set_hook(_ntff_profile_via_ctypes("/opt/axon/libaxon_pjrt.so"))
        except Exception:
            pass
    except ImportError:
        pass

import concourse.bass as bass  # noqa: E402
import concourse.mybir as mybir  # noqa: E402
import concourse.tile as tile  # noqa: E402
from concourse import bacc  # noqa: E402
from concourse.bass_utils import run_bass_kernel_spmd  # noqa: E402

N_CORES = 8
B, C, H, W = 4, 64, 128, 128
HEADS, DIM_HEAD = 64, 64
HW = H * W
JS = HW // N_CORES  # 2048 j-positions per core
SCALE = DIM_HEAD ** -0.5
F32 = mybir.dt.float32
BF16 = mybir.dt.bfloat16
NPBF16 = ml_dtypes.bfloat16

_compiled = None  # cache (nc) across calls


def _build():
    nc = bacc.Bacc("TRN2", target_bir_lowering=False, debug=False,
                   num_devices=N_CORES)

    fimg_d = nc.dram_tensor("fimg_s", [2 * 128, JS], BF16, kind="ExternalInput")
    wqf2_d = nc.dram_tensor("WqF2", [128, 2056], BF16, kind="ExternalInput")
    wk_d = nc.dram_tensor("Wk_st", [128, 2048], BF16, kind="ExternalInput")
    out_d = nc.dram_tensor("out", [2 * 128, JS], BF16, kind="ExternalOutput")

    with tile.TileContext(nc) as tc:
        with (
            tc.tile_pool(name="weights", bufs=1) as wpool,
            tc.tile_pool(name="img", bufs=1) as ipool,
            tc.tile_pool(name="small_ps", bufs=1, space="PSUM") as spsum,
            tc.tile_pool(name="big_ps", bufs=6, space="PSUM") as bpsum,
            tc.tile_pool(name="ostage", bufs=1) as opool,
        ):
            # Zero-init tiles: no input deps, runs during the NEFF entry
            # sequence / first DMA wait. All on gpsimd (SBUF-only engine,
            # otherwise idle); the one-time cross-engine WAW chain
            # resolves long before the readers run.
            q2b = []
            for t in range(2):
                tt = wpool.tile([128, 128], BF16, tag=f"q2b{t}")
                nc.gpsimd.memset(tt[:], 0.0)
                q2b.append(tt)
            bds = []
            for q in range(2):
                bd = wpool.tile([128, 128], BF16, tag=f"bd{q}")
                nc.gpsimd.memset(bd[:], 0.0)
                bds.append(bd)
            zeros = wpool.tile([128, 128], BF16, tag="zeros")
            nc.gpsimd.memset(zeros[:], 0.0)

            # Input DMAs on two HWDGE queues (sync + scalar), which share
            # ~295 GB/s aggregate. All transfers keep 4KB+ per-partition
            # rows (smaller rows tank packet throughput). The two weight
            # tensors are split by partition halves across BOTH queues so
            # the W2 chain's inputs land as early as possible.
            wqf2 = wpool.tile([128, 2056], BF16, tag="wqf2")
            nc.sync.dma_start(wqf2[:], wqf2_d.ap())
            wk = wpool.tile([128, 2048], BF16, tag="wk")
            nc.scalar.dma_start(wk[:], wk_d.ap())
            imgs = []
            for q in range(2):
                img_t = ipool.tile([128, JS], BF16, tag=f"img{q}")
                eng = nc.sync if q == 0 else nc.scalar
                eng.dma_start(img_t[:], fimg_d.ap()[128 * q:128 * (q + 1), :])
                imgs.append(img_t)

            # A: q2T[r, 4m+b] = q[b, 128m + r]; chunk pairs m=2p,2p+1
            # stacked along partitions, fpsf block-diag-expanded -> N=8.
            # Two separate PSUM tiles (t = m//16) so the expansion for the
            # first half doesn't wait for all of A (deps are tracked at
            # tile granularity).
            fpsf_exp = wqf2[:, 0:8]
            q2T_ps = spsum.tile([128, 128], F32, tag="q2T_ps")
            # PE clock-gate (HAM) warm-up: the PE boots throttled to
            # 1.2 GHz and needs several us of sustained matmul activity
            # to unthrottle. Dummy matmuls keep it busy through the DMA
            # wait and every dependency gap so the big matmuls (the
            # longest PE phase) run at full clock. Targets are PSUM
            # tiles whose real contents are written afterwards.
            for _ in range(28):
                nc.tensor.matmul(q2T_ps[:], zeros[:], zeros[:],
                                 start=True, stop=True)
            for p in range(16):
                nc.tensor.matmul(
                    q2T_ps[:, 8 * p:8 * p + 8],
                    wqf2[:, 8 + 128 * p:8 + 128 * p + 128],
                    fpsf_exp,
                    start=True, stop=True,
                )

            # Zero-expand q2T for B (scale folded in): col 8pp+2b+par
            # holds q[b, head 2p+par] in rows 64par:64par+64, zeros
            # elsewhere. All on vector (scalar on this chain measures
            # ~3us slower: scheduler reordering + handoff latency); two
            # tiles so B's first half only waits for the t=0 ops.
            for t in range(2):
                for par in range(2):
                    dst = q2b[t][64 * par:64 * par + 64, par:128:2]
                    src = q2T_ps[64 * par:64 * par + 64, 64 * t:64 * t + 64]
                    nc.vector.tensor_scalar_mul(dst, src, SCALE)

            # B: w2_ps[c, 8p + 2b + par] = scale * W2[b, 2p+par, c]
            w2_ps = spsum.tile([64, 256], F32, tag="w2_ps")
            for _ in range(10):
                nc.tensor.matmul(w2_ps[:, 0:128], zeros[:, 0:64], zeros[:],
                                 start=True, stop=True)
            for p in range(32):
                t, pp = divmod(p, 16)
                nc.tensor.matmul(
                    w2_ps[:, 8 * p:8 * p + 8],
                    wk[:, 64 * p:64 * p + 64],
                    q2b[t][:, 8 * pp:8 * pp + 8],
                    start=True, stop=True,
                )

            # Assembly: bd_q[64*half + c, 64*half + 2p+par] =
            # sW2[2q+half, 2p+par, c]. One merged 3D-AP copy per (q,
            # half): src w2[c, 8p + 2b + par], dst bd[c, 2p + par].
            # bd0 on scalar (idle at this point), bd1 on vector — one
            # writer engine per tile, both start right after B.
            # Fillers covering the assembly window (q2T_ps is free again:
            # the expansion's reads are done before B finishes).
            for _ in range(12):
                nc.tensor.matmul(q2T_ps[:], zeros[:], zeros[:],
                                 start=True, stop=True)

            w2_3d = w2_ps[:, :].rearrange("c (p x) -> c p x", p=32)
            for q in range(2):
                for half in range(2):
                    b = 2 * q + half
                    dst = bds[q][64 * half:64 * half + 64,
                                 64 * half:64 * half + 64].rearrange(
                                     "c (p x) -> c p x", p=32)
                    src = w2_3d[:, :, 2 * b:2 * b + 2]
                    if q == 0:
                        nc.scalar.copy(dst, src)
                    else:
                        nc.vector.tensor_copy(dst, src)

            # Big: out rows pair q = bd_q.T @ img_q in 512-col chunks.
            # Stage per (q, half) into [128, 1024] bf16 tiles, each with a
            # single writer engine (h0 -> vector, h1 -> scalar), and DMA
            # each half as soon as its two casts land (4 out DMAs total,
            # 2KB rows, alternating queues).
            for q in range(2):
                for half in range(2):
                    ot = opool.tile([128, JS // 2], BF16, tag=f"ot{q}{half}")
                    for kk in range(2):
                        k = 2 * half + kk
                        ps = bpsum.tile([128, 512], F32, tag="mm_ps")
                        nc.tensor.matmul(
                            ps[:], bds[q][:],
                            imgs[q][:, 512 * k:512 * k + 512],
                            start=True, stop=True,
                        )
                        dst = ot[:, 512 * kk:512 * kk + 512]
                        if half == 0:
                            nc.vector.tensor_copy(dst, ps[:])
                        else:
                            nc.scalar.copy(dst, ps[:])
                    (nc.sync if half == 0 else nc.scalar).dma_start(
                        out_d.ap()[128 * q:128 * (q + 1),
                                   1024 * half:1024 * half + 1024],
                        ot[:])

    nc.compile()
    return nc


def _prep_inputs(fpsf, fimg, Wq, Wk):
    fpsf = np.ascontiguousarray(fpsf, dtype=np.float32)
    fimg = np.ascontiguousarray(fimg, dtype=np.float32)
    Wq = np.ascontiguousarray(Wq, dtype=np.float32)
    Wk = np.ascontiguousarray(Wk, dtype=np.float32)

    WqT3 = Wq.T.reshape(64, 32, 128)  # [e, chunk, s]
    wqf2 = np.zeros((128, 2056), np.float32)
    wqf2[0:64, 0:4] = fpsf.T
    wqf2[64:128, 4:8] = fpsf.T
    wqf2[0:64, 8:] = WqT3[:, 0::2, :].reshape(64, 2048)
    wqf2[64:128, 8:] = WqT3[:, 1::2, :].reshape(64, 2048)
    WqF2 = wqf2.astype(NPBF16)

    Wk3 = Wk.reshape(64, 64, 64)  # [h, d, c]
    wk = np.empty((128, 2048), np.float32)
    wk[0:64] = Wk3[0::2].transpose(1, 0, 2).reshape(64, 2048)
    wk[64:128] = Wk3[1::2].transpose(1, 0, 2).reshape(64, 2048)
    Wk_st = wk.astype(NPBF16)

    fimg_f = fimg.reshape(B, C, HW).astype(NPBF16)
    in_maps = []
    for i in range(N_CORES):
        sh = np.ascontiguousarray(
            fimg_f[:, :, JS * i:JS * (i + 1)]).reshape(2 * 128, JS)
        in_maps.append({
            "fimg_s": sh,
            "WqF2": WqF2,
            "Wk_st": Wk_st,
        })
    return in_maps


def kernel(fpsf, fimg, Wq, Wk):
    global _compiled
    if _compiled is None:
        _compiled = _build()
    nc = _compiled

    in_maps = _prep_inputs(fpsf, fimg, Wq, Wk)
    res = run_bass_kernel_spmd(nc, in_maps, core_ids=list(range(N_CORES)))

    out = np.empty((B, HEADS, HW), dtype=np.float32)
    for i in range(N_CORES):
        out[:, :, JS * i:JS * (i + 1)] = \
            res.results[i]["out"].astype(np.float32).reshape(B, HEADS, JS)
    return out.reshape(B, C, H, W)


if __name__ == "__main__":
    rng = np.random.default_rng(0)
    ins = {
        "fpsf": rng.standard_normal((B, C), dtype=np.float32),
        "fimg": rng.standard_normal((B, C, H, W), dtype=np.float32),
        "Wq": (rng.standard_normal((4096, C), dtype=np.float32) * 0.05),
        "Wk": (rng.standard_normal((4096, C), dtype=np.float32) * 0.05),
    }
    out = kernel(**ins)
    print("out", out.shape, out.dtype, float(np.abs(out).max()))
